# revision 36
# baseline (speedup 1.0000x reference)
"""DenseGINEConv on 8 TRN2 NeuronCores (Bass/Tile).

Reference computation (B=4, N=512, F=64, H=128):
    msg  = leaky_relu(adj[b,i,j] * (x[b,i,f] + edge_attr[b,i,j,f]), 0.01)
    agg  = sum_i msg                         # (B, N, F) indexed by destination j
    out  = x + agg
    h    = leaky_relu(out @ W1 + b1) @ W2 + b2
    res  = where(mask[b,j], h, 0)

Key facts used:
  * adj >= 0 (uniform fill), so leaky_relu(adj*z) = adj * leaky_relu(z).
    The adj multiply + i-reduction then fuse into ONE TensorE matmul per
    JG-wide destination-node group: cross[j,(j',f)] = sum_i adj[i,j]*u[i,(j',f)],
    of which the block diagonal is kept via a mask-multiply + strided reduce.
  * Rows with mask=0 produce zero output, so each core only processes its
    compacted list of kept destination nodes (j-compaction on the host); the
    host scatter keeps only the first J rows per core, so no on-device mask
    is needed at all.

Sharding: core c = 2*b + h handles batch b and half of b's kept destination
nodes (interleaved for balance). Sum over source axis i stays local; no
collectives. Each core returns a dense [Jp, F] block that the host scatters
back into the full (B, N, F) output.

Pipeline per core: groups of JG=16 destination nodes, processed in pairs
(one 1 MB DMA per (pair, i-block)). The big elementwise add alternates
between VectorE and GPSIMD; LeakyReLU runs on ScalarE (output rounded to
fp32r); the adjacency contraction runs on TensorE in fp32r. Each pair's
MLP tail is pipelined right after its aggregation so only the final
pair's tiny MLP sits after the last DMA.
"""
import numpy as np

import concourse.bacc as bacc
import concourse.mybir as mybir
import concourse.tile as tile
from concourse.bass_utils import run_bass_kernel_spmd

B, N, F, H = 4, 512, 64, 128
NEG_SLOPE = 0.01
P = 128          # partitions / i-block size
NI = N // P      # number of i blocks (4)
JG = 12          # destination-node group size


def _chunks():
    """Split the per-group free width JG*F into matmul-N chunks (<=512,
    >=256 so fp32r runs at full rate, bank-aligned)."""
    total = JG * F
    out = []
    off = 0
    while total - off > 512:
        out.append((off, 512))
        off += 512
    out.append((off, total - off))
    return out


N_CHUNKS = _chunks()
N_CORES = 8

F32 = mybir.dt.float32
F32R = mybir.dt.float32r

_PROG_CACHE = {}


def _const_layout(G):
    """Column layout of the packed [P, CW] constant tensor."""
    cols = {}
    off = 0
    for name, width in [("dm", JG * F),
                        ("w1", H), ("w2", F), ("b1", 1), ("b2", 1),
                        ("ident", P)]:
        cols[name] = (off, width)
        off += width
    return cols, off


def _build(Jp: int, widths=None, e_bufs=4, z_bufs=6, u_bufs=4, cross_bufs=3,
           dve_tail_pairs=0, add_dve_every=2):
    """Build the per-core Bass program for a padded kept-j count of Jp."""
    assert Jp % JG == 0
    G = Jp // JG
    cols, CW = _const_layout(G)
    nc = bacc.Bacc("TRN2", target_bir_lowering=False)

    edge_d = nc.dram_tensor("edge", [N, Jp, F], F32, kind="ExternalInput")
    x_d = nc.dram_tensor("x", [P, NI * F], F32, kind="ExternalInput")
    adj_d = nc.dram_tensor("adj", [P, NI * Jp], F32R, kind="ExternalInput")
    # single-partition payload: ones[JG] ++ xk.flat [Jp*F]; used to fold
    # "+ x_j" into the PE accumulation as a K=1 matmul
    xtr_d = nc.dram_tensor("xtr", [1, JG + Jp * F], F32R, kind="ExternalInput")
    cst_d = nc.dram_tensor("cst", [P, CW], F32, kind="ExternalInput")
    out_d = nc.dram_tensor("out", [Jp, F], F32, kind="ExternalOutput")

    with tile.TileContext(nc) as tc:
        with tc.tile_pool(name="cpool", bufs=1) as cpool:
            # x loads first (tiny) so the z-prefill copies start immediately
            xs_t = cpool.tile([P, NI * F], F32)
            nc.sync.dma_start(out=xs_t[:, :], in_=x_d[:, :])
            c_t = cpool.tile([P, CW], F32)
            adj_t = cpool.tile([P, NI * Jp], F32R)
            xtr_t = cpool.tile([1, JG + Jp * F], F32R)

            def load_consts():
                # issued after the first edge DMA so the stream starts ASAP;
                # the first matmul (which needs adj) comes later anyway
                nc.sync.dma_start(out=c_t[:, :], in_=cst_d[:, :])
                nc.sync.dma_start(out=adj_t[:, :], in_=adj_d[:, :])
                nc.sync.dma_start(out=xtr_t[:, :], in_=xtr_d[:, :])

            def cslice(name):
                o, w = cols[name]
                return c_t[:, o:o + w]

            x_t = xs_t[:, :].rearrange("p (ib f) -> p ib f", ib=NI)
            dm_t = cslice("dm")[:JG, :]
            w1_t = cslice("w1")[:F, :]
            w2_t = cslice("w2")[:H, :]
            b1_t = cslice("b1")[:H, :]
            b2_t = cslice("b2")[:F, :]
            id_t = cslice("ident")
            adj_v = adj_t[:, :].rearrange("p (ib j) -> p ib j", ib=NI)
            ones_r = xtr_t[0:1, :JG]
            xk_r = xtr_t[0:1, JG:]

            # group pairs [g0, g0+W): wide early, width-1 at the end so the
            # post-last-DMA dependency chains are as short as possible
            if widths is None:
                widths = [2] * ((G - 1) // 2) + [1] * (1 + (G - 1) % 2)
            assert sum(widths) == G
            pairs = []
            g = 0
            for w in widths:
                pairs.append((g, w))
                g += w
            MAXW = max(widths)

            with tc.tile_pool(name="spool", bufs=2) as spool, \
                 tc.tile_pool(name="pstream", bufs=1, space="PSUM") as pstream:
                add_i = 0
                for pi, (g0, W) in enumerate(pairs):
                    is_tail = pi >= len(pairs) - dve_tail_pairs
                    JW = W * JG                 # nodes in this pair
                    FW = JW * F                 # free width of stream tiles
                    crs = [pstream.tile([JG, JG * F], F32, tag="cross",
                                        bufs=cross_bufs,
                                        name=f"cross_g{g0 + gi}")
                           for gi in range(W)]
                    for ib in range(NI):
                        # z = broadcast(x[ib]) filled by DVE (2x-mode copy),
                        # then the edge tile is DMA'd on top with the DMA
                        # engines' inline CCE adder: z += e. The big
                        # elementwise add costs no vector-engine time.
                        z_t = spool.tile([P, FW], F32, tag="z", bufs=z_bufs,
                                         padded_shape=[P, MAXW * JG * F])
                        x_b = x_t[:, ib:ib + 1, :].broadcast_to([P, JW, F])
                        nc.vector.tensor_copy(z_t[:, :], x_b)
                        nc.gpsimd.dma_start(
                            out=z_t[:, :],
                            in_=edge_d[ib * P:(ib + 1) * P,
                                       g0 * JG:g0 * JG + JW, :],
                            accum_op=mybir.AluOpType.add)
                        if pi == 0 and ib == 0:
                            load_consts()
                        u_t = spool.tile([P, FW], F32R, tag="u", bufs=u_bufs,
                                         padded_shape=[P, MAXW * JG * F])
                        nc.scalar.activation(u_t[:, :], z_t[:, :],
                                             mybir.ActivationFunctionType.Lrelu,
                                             alpha=NEG_SLOPE)
                        for gi in range(W):
                            lhsT = adj_v[:, ib,
                                         (g0 + gi) * JG:(g0 + gi + 1) * JG]
                            for (co, cw) in N_CHUNKS:
                                nc.tensor.matmul(
                                    crs[gi][:, co:co + cw],
                                    lhsT,
                                    u_t[:, gi * JG * F + co:
                                        gi * JG * F + co + cw],
                                    start=(ib == 0), stop=False)
                    # K=1 matmul folds "+ xk" into the accumulated cross so
                    # the diagonal reduce directly yields agg + xk
                    for gi in range(W):
                        g = g0 + gi
                        for (co, cw) in N_CHUNKS:
                            nc.tensor.matmul(
                                crs[gi][:, co:co + cw],
                                ones_r,
                                xk_r[:, g * JG * F + co:
                                     g * JG * F + co + cw],
                                start=False, stop=True)

                    # diagonal extraction for each group in the pair
                    o_t = spool.tile([JG, W, F], F32, tag="o",
                                     padded_shape=[JG, MAXW, F])
                    for gi in range(W):
                        # mask-mult + strided reduce, split into j'-halves so
                        # the reduce of half 0 overlaps the mult of half 1
                        stage = spool.tile([JG, JG * F], F32, tag="stage",
                                           name=f"stage_g{g0 + gi}")
                        hj = JG // 2
                        for h0 in range(2):
                            sl = slice(h0 * hj * F, (h0 + hj * (1 - h0) + hj * h0) * 0 + (h0 + 1) * hj * F)
                            nc.vector.tensor_tensor(
                                out=stage[:, sl], in0=crs[gi][:, sl],
                                in1=dm_t[:, sl], op=mybir.AluOpType.mult)
                            stage_v = stage[:, sl].rearrange(
                                "p (j f) -> p j f", j=hj).transpose([0, 2, 1])
                            acc = o_t[:, gi, :]
                            if h0 == 0:
                                nc.vector.reduce_sum(acc, stage_v,
                                                     axis=mybir.AxisListType.X)
                            else:
                                half = spool.tile([JG, F], F32, tag="ohalf")
                                nc.vector.reduce_sum(half[:, :], stage_v,
                                                     axis=mybir.AxisListType.X)
                                nc.vector.tensor_tensor(
                                    out=acc, in0=acc, in1=half[:, :],
                                    op=mybir.AluOpType.add)

                    # pair tail: h = lrelu(o@W1+b1)@W2+b2  (o already has +xk)
                    outT_p = pstream.tile([F, JW], F32, tag="mlp", bufs=2,
                                          padded_shape=[F, MAXW * JG])
                    for gi in range(W):
                        nc.tensor.transpose(outT_p[:, gi * JG:(gi + 1) * JG],
                                            o_t[:, gi, :], id_t[:JG, :JG])
                    outT_s = spool.tile([F, JW], F32, tag="outT",
                                        padded_shape=[F, MAXW * JG])
                    nc.scalar.copy(outT_s[:, :], outT_p[:, :])

                    h_p = pstream.tile([H, JW], F32, tag="mlp", bufs=2,
                                       padded_shape=[H, MAXW * JG])
                    nc.tensor.matmul(h_p[:, :], w1_t[:, :], outT_s[:, :],
                                     start=True, stop=True)
                    h_s = spool.tile([H, JW], F32, tag="h",
                                     padded_shape=[H, MAXW * JG])
                    nc.scalar.activation(h_s[:, :], h_p[:, :],
                                         mybir.ActivationFunctionType.Lrelu,
                                         bias=b1_t, alpha=NEG_SLOPE)

                    y_p = pstream.tile([F, JW], F32, tag="mlp", bufs=2,
                                       padded_shape=[F, MAXW * JG])
                    nc.tensor.matmul(y_p[:, :], w2_t[:, :], h_s[:, :],
                                     start=True, stop=True)
                    y_s = spool.tile([F, JW], F32, tag="y",
                                     padded_shape=[F, MAXW * JG])
                    nc.scalar.activation(y_s[:, :], y_p[:, :],
                                         mybir.ActivationFunctionType.Identity,
                                         bias=b2_t)

                    yT_p = pstream.tile([JG, W * F], F32, tag="mlp", bufs=2,
                                        padded_shape=[JG, MAXW * F])
                    for gi in range(W):
                        nc.tensor.transpose(yT_p[:, gi * F:(gi + 1) * F],
                                            y_s[:, gi * JG:(gi + 1) * JG],
                                            id_t[:F, :F])
                    yT_s = spool.tile([JG, W * F], F32, tag="yT",
                                      padded_shape=[JG, MAXW * F])
                    nc.vector.tensor_copy(yT_s[:, :], yT_p[:, :])
                    nc.sync.dma_start(
                        out=out_d[g0 * JG:g0 * JG + JW, :].rearrange(
                            "(g p) f -> p g f", p=JG),
                        in_=yT_s[:, :].rearrange("p (g f) -> p g f", g=W))

    nc.compile()
    return nc


def _get_prog(Jp: int):
    if Jp not in _PROG_CACHE:
        _PROG_CACHE[Jp] = _build(Jp)
    return _PROG_CACHE[Jp]


def _pack_consts(Jp, W1, W2, b1, b2):
    G = Jp // JG
    cols, CW = _const_layout(G)
    cst = np.zeros((P, CW), np.float32)

    def put(name, arr):
        o, w = cols[name]
        cst[:arr.shape[0], o:o + w] = arr

    dm = np.kron(np.eye(JG, dtype=np.float32), np.ones((1, F), np.float32))
    put("dm", dm)
    put("w1", W1)
    put("w2", W2)
    put("b1", b1[:, None])
    put("b2", b2[:, None])
    put("ident", np.eye(P, dtype=np.float32))
    return cst


def kernel(x, adj, edge_attr, mask, W1, b1, W2, b2):
    x = np.ascontiguousarray(np.asarray(x, dtype=np.float32))
    adj = np.ascontiguousarray(np.asarray(adj, dtype=np.float32))
    edge_attr = np.ascontiguousarray(np.asarray(edge_attr, dtype=np.float32))
    mask = np.asarray(mask)
    W1 = np.ascontiguousarray(np.asarray(W1, dtype=np.float32))
    b1 = np.ascontiguousarray(np.asarray(b1, dtype=np.float32))
    W2 = np.ascontiguousarray(np.asarray(W2, dtype=np.float32))
    b2 = np.ascontiguousarray(np.asarray(b2, dtype=np.float32))

    # core c = 2*b + h: batch b, interleaved half h of b's kept nodes
    core_jj = []
    for b in range(B):
        jj = np.flatnonzero(mask[b])
        core_jj.append(jj[0::2])
        core_jj.append(jj[1::2])
    maxJ = max((len(jj) for jj in core_jj), default=1)
    Jp = max(JG, ((maxJ + JG - 1) // JG) * JG)

    nc = _get_prog(Jp)

    in_maps = []
    for c, jj in enumerate(core_jj):
        b = c // 2
        J = len(jj)
        edge_c = np.zeros((N, Jp, F), np.float32)
        if J:
            edge_c[:, :J] = edge_attr[b][:, jj, :]
        adj_c = np.zeros((N, Jp), np.float32)
        if J:
            adj_c[:, :J] = adj[b][:, jj]
        xk = np.zeros((Jp, F), np.float32)
        if J:
            xk[:J] = x[b][jj]
        adj_r = adj_c.reshape(NI, P, Jp).transpose(1, 0, 2).reshape(P, NI * Jp)
        xtr = np.concatenate([np.ones(JG, np.float32), xk.reshape(-1)])[None, :]
        cst = _pack_consts(Jp, W1, W2, b1, b2)
        x_r = x[b].reshape(NI, P, F).transpose(1, 0, 2).reshape(P, NI * F)
        in_maps.append({
            "edge": edge_c, "adj": np.ascontiguousarray(adj_r),
            "xtr": np.ascontiguousarray(xtr), "cst": cst,
            "x": np.ascontiguousarray(x_r),
        })

    res = run_bass_kernel_spmd(nc, in_maps, list(range(N_CORES)))

    out = np.zeros((B, N, F), np.float32)
    for c, jj in enumerate(core_jj):
        b = c // 2
        if len(jj):
            out[b][jj] = res.results[c]["out"][:len(jj)]
    return out


# revision 37
# speedup vs baseline: 1.0307x; 1.0307x over previous
"""DenseGINEConv on 8 TRN2 NeuronCores (Bass/Tile).

Reference computation (B=4, N=512, F=64, H=128):
    msg  = leaky_relu(adj[b,i,j] * (x[b,i,f] + edge_attr[b,i,j,f]), 0.01)
    agg  = sum_i msg                         # (B, N, F) indexed by destination j
    out  = x + agg
    h    = leaky_relu(out @ W1 + b1) @ W2 + b2
    res  = where(mask[b,j], h, 0)

Key facts used:
  * adj >= 0 (uniform fill), so leaky_relu(adj*z) = adj * leaky_relu(z).
    The adj multiply + i-reduction then fuse into ONE TensorE matmul per
    JG-wide destination-node group: cross[j,(j',f)] = sum_i adj[i,j]*u[i,(j',f)],
    of which the block diagonal is kept via a mask-multiply + strided reduce.
  * Rows with mask=0 produce zero output, so each core only processes its
    compacted list of kept destination nodes (j-compaction on the host); the
    host scatter keeps only the first J rows per core, so no on-device mask
    is needed at all.

Sharding: core c = 2*b + h handles batch b and half of b's kept destination
nodes (interleaved for balance). Sum over source axis i stays local; no
collectives. Each core returns a dense [Jp, F] block that the host scatters
back into the full (B, N, F) output.

Pipeline per core: groups of JG=16 destination nodes, processed in pairs
(one 1 MB DMA per (pair, i-block)). The big elementwise add alternates
between VectorE and GPSIMD; LeakyReLU runs on ScalarE (output rounded to
fp32r); the adjacency contraction runs on TensorE in fp32r. Each pair's
MLP tail is pipelined right after its aggregation so only the final
pair's tiny MLP sits after the last DMA.
"""
import numpy as np

import concourse.bacc as bacc
import concourse.mybir as mybir
import concourse.tile as tile
from concourse.bass_utils import run_bass_kernel_spmd

B, N, F, H = 4, 512, 64, 128
NEG_SLOPE = 0.01
P = 128          # partitions / i-block size
NI = N // P      # number of i blocks (4)
JG = 12          # destination-node group size


def _chunks():
    """Split the per-group free width JG*F into matmul-N chunks (<=512,
    >=256 so fp32r runs at full rate, bank-aligned)."""
    total = JG * F
    out = []
    off = 0
    while total - off > 512:
        out.append((off, 512))
        off += 512
    out.append((off, total - off))
    return out


N_CHUNKS = _chunks()
N_CORES = 8

F32 = mybir.dt.float32
F32R = mybir.dt.float32r

_PROG_CACHE = {}


def _const_layout(G):
    """Column layout of the packed [P, CW] constant tensor."""
    cols = {}
    off = 0
    for name, width in [("dm", JG * F),
                        ("w1", H), ("w2", F), ("b1", 1), ("b2", 1),
                        ("ident", P)]:
        cols[name] = (off, width)
        off += width
    return cols, off


def _build(Jp: int, widths=None, e_bufs=4, z_bufs=6, u_bufs=4, cross_bufs=3,
           dve_tail_pairs=0, add_dve_every=2):
    """Build the per-core Bass program for a padded kept-j count of Jp."""
    assert Jp % JG == 0
    G = Jp // JG
    cols, CW = _const_layout(G)
    nc = bacc.Bacc("TRN2", target_bir_lowering=False)

    edge_d = nc.dram_tensor("edge", [N, Jp, F], F32, kind="ExternalInput")
    x_d = nc.dram_tensor("x", [P, NI * F], F32, kind="ExternalInput")
    adj_d = nc.dram_tensor("adj", [P, NI * Jp], F32R, kind="ExternalInput")
    # single-partition payload: ones[JG] ++ xk.flat [Jp*F]; used to fold
    # "+ x_j" into the PE accumulation as a K=1 matmul
    xtr_d = nc.dram_tensor("xtr", [1, JG + Jp * F], F32R, kind="ExternalInput")
    cst_d = nc.dram_tensor("cst", [P, CW], F32, kind="ExternalInput")
    out_d = nc.dram_tensor("out", [Jp, F], F32, kind="ExternalOutput")

    with tile.TileContext(nc) as tc:
        with tc.tile_pool(name="cpool", bufs=1) as cpool:
            # x loads first (tiny) so the z-prefill copies start immediately
            xs_t = cpool.tile([P, NI * F], F32)
            nc.sync.dma_start(out=xs_t[:, :], in_=x_d[:, :])
            c_t = cpool.tile([P, CW], F32)
            adj_t = cpool.tile([P, NI * Jp], F32R)
            xtr_t = cpool.tile([1, JG + Jp * F], F32R)

            def load_consts():
                # issued after the first edge DMA so the stream starts ASAP;
                # the first matmul (which needs adj) comes later anyway
                nc.sync.dma_start(out=c_t[:, :], in_=cst_d[:, :])
                nc.sync.dma_start(out=adj_t[:, :], in_=adj_d[:, :])
                nc.sync.dma_start(out=xtr_t[:, :], in_=xtr_d[:, :])

            def cslice(name):
                o, w = cols[name]
                return c_t[:, o:o + w]

            x_t = xs_t[:, :].rearrange("p (ib f) -> p ib f", ib=NI)
            dm_t = cslice("dm")[:JG, :]
            w1_t = cslice("w1")[:F, :]
            w2_t = cslice("w2")[:H, :]
            b1_t = cslice("b1")[:H, :]
            b2_t = cslice("b2")[:F, :]
            id_t = cslice("ident")
            adj_v = adj_t[:, :].rearrange("p (ib j) -> p ib j", ib=NI)
            ones_r = xtr_t[0:1, :JG]
            xk_r = xtr_t[0:1, JG:]

            # group pairs [g0, g0+W): wide early, width-1 at the end so the
            # post-last-DMA dependency chains are as short as possible
            if widths is None:
                widths = [2] * ((G - 1) // 2) + [1] * (1 + (G - 1) % 2)
            assert sum(widths) == G
            pairs = []
            g = 0
            for w in widths:
                pairs.append((g, w))
                g += w
            MAXW = max(widths)

            with tc.tile_pool(name="spool", bufs=2) as spool, \
                 tc.tile_pool(name="pstream", bufs=1, space="PSUM") as pstream:
                add_i = 0
                for pi, (g0, W) in enumerate(pairs):
                    is_tail = pi >= len(pairs) - dve_tail_pairs
                    JW = W * JG                 # nodes in this pair
                    FW = JW * F                 # free width of stream tiles
                    crs = [pstream.tile([JG, JG * F], F32, tag="cross",
                                        bufs=cross_bufs,
                                        name=f"cross_g{g0 + gi}")
                           for gi in range(W)]
                    for ib in range(NI):
                        # z = broadcast(x[ib]) filled by DVE (2x-mode copy),
                        # then the edge tile is DMA'd on top with the DMA
                        # engines' inline CCE adder: z += e. The big
                        # elementwise add costs no vector-engine time.
                        z_t = spool.tile([P, FW], F32, tag="z", bufs=z_bufs,
                                         padded_shape=[P, MAXW * JG * F])
                        x_b = x_t[:, ib:ib + 1, :].broadcast_to([P, JW, F])
                        nc.vector.tensor_copy(z_t[:, :], x_b)
                        nc.gpsimd.dma_start(
                            out=z_t[:, :],
                            in_=edge_d[ib * P:(ib + 1) * P,
                                       g0 * JG:g0 * JG + JW, :],
                            accum_op=mybir.AluOpType.add)
                        if pi == 0 and ib == 0:
                            load_consts()
                        u_t = spool.tile([P, FW], F32R, tag="u", bufs=u_bufs,
                                         padded_shape=[P, MAXW * JG * F])
                        nc.scalar.activation(u_t[:, :], z_t[:, :],
                                             mybir.ActivationFunctionType.Lrelu,
                                             alpha=NEG_SLOPE)
                        for gi in range(W):
                            lhsT = adj_v[:, ib,
                                         (g0 + gi) * JG:(g0 + gi + 1) * JG]
                            for (co, cw) in N_CHUNKS:
                                nc.tensor.matmul(
                                    crs[gi][:, co:co + cw],
                                    lhsT,
                                    u_t[:, gi * JG * F + co:
                                        gi * JG * F + co + cw],
                                    start=(ib == 0), stop=False)
                    # K=1 matmul folds "+ xk" into the accumulated cross so
                    # the diagonal reduce directly yields agg + xk
                    for gi in range(W):
                        g = g0 + gi
                        for (co, cw) in N_CHUNKS:
                            nc.tensor.matmul(
                                crs[gi][:, co:co + cw],
                                ones_r,
                                xk_r[:, g * JG * F + co:
                                     g * JG * F + co + cw],
                                start=False, stop=True)

                    # diagonal extraction for each group in the pair
                    o_t = spool.tile([JG, W, F], F32, tag="o",
                                     padded_shape=[JG, MAXW, F])
                    for gi in range(W):
                        stage = spool.tile([JG, JG * F], F32, tag="stage",
                                           name=f"stage_g{g0 + gi}")
                        nc.vector.tensor_tensor(
                            out=stage[:, :], in0=crs[gi][:, :],
                            in1=dm_t[:, :], op=mybir.AluOpType.mult)
                        stage_v = stage[:, :].rearrange(
                            "p (j f) -> p j f", j=JG).transpose([0, 2, 1])
                        nc.vector.reduce_sum(o_t[:, gi, :], stage_v,
                                             axis=mybir.AxisListType.X)

                    # pair tail: h = lrelu(o@W1+b1)@W2+b2  (o already has +xk)
                    outT_p = pstream.tile([F, JW], F32, tag="mlp", bufs=2,
                                          padded_shape=[F, MAXW * JG])
                    for gi in range(W):
                        nc.tensor.transpose(outT_p[:, gi * JG:(gi + 1) * JG],
                                            o_t[:, gi, :], id_t[:JG, :JG])
                    outT_s = spool.tile([F, JW], F32, tag="outT",
                                        padded_shape=[F, MAXW * JG])
                    nc.scalar.copy(outT_s[:, :], outT_p[:, :])

                    h_p = pstream.tile([H, JW], F32, tag="mlp", bufs=2,
                                       padded_shape=[H, MAXW * JG])
                    nc.tensor.matmul(h_p[:, :], w1_t[:, :], outT_s[:, :],
                                     start=True, stop=True)
                    h_s = spool.tile([H, JW], F32, tag="h",
                                     padded_shape=[H, MAXW * JG])
                    nc.scalar.activation(h_s[:, :], h_p[:, :],
                                         mybir.ActivationFunctionType.Lrelu,
                                         bias=b1_t, alpha=NEG_SLOPE)

                    y_p = pstream.tile([F, JW], F32, tag="mlp", bufs=2,
                                       padded_shape=[F, MAXW * JG])
                    nc.tensor.matmul(y_p[:, :], w2_t[:, :], h_s[:, :],
                                     start=True, stop=True)
                    y_s = spool.tile([F, JW], F32, tag="y",
                                     padded_shape=[F, MAXW * JG])
                    nc.scalar.activation(y_s[:, :], y_p[:, :],
                                         mybir.ActivationFunctionType.Identity,
                                         bias=b2_t)

                    yT_p = pstream.tile([JG, W * F], F32, tag="mlp", bufs=2,
                                        padded_shape=[JG, MAXW * F])
                    for gi in range(W):
                        nc.tensor.transpose(yT_p[:, gi * F:(gi + 1) * F],
                                            y_s[:, gi * JG:(gi + 1) * JG],
                                            id_t[:F, :F])
                    yT_s = spool.tile([JG, W * F], F32, tag="yT",
                                      padded_shape=[JG, MAXW * F])
                    nc.vector.tensor_copy(yT_s[:, :], yT_p[:, :])
                    nc.sync.dma_start(
                        out=out_d[g0 * JG:g0 * JG + JW, :].rearrange(
                            "(g p) f -> p g f", p=JG),
                        in_=yT_s[:, :].rearrange("p (g f) -> p g f", g=W))

    nc.compile()
    return nc


def _get_prog(Jp: int):
    if Jp not in _PROG_CACHE:
        _PROG_CACHE[Jp] = _build(Jp)
    return _PROG_CACHE[Jp]


def _pack_consts(Jp, W1, W2, b1, b2):
    G = Jp // JG
    cols, CW = _const_layout(G)
    cst = np.zeros((P, CW), np.float32)

    def put(name, arr):
        o, w = cols[name]
        cst[:arr.shape[0], o:o + w] = arr

    dm = np.kron(np.eye(JG, dtype=np.float32), np.ones((1, F), np.float32))
    put("dm", dm)
    put("w1", W1)
    put("w2", W2)
    put("b1", b1[:, None])
    put("b2", b2[:, None])
    put("ident", np.eye(P, dtype=np.float32))
    return cst


def kernel(x, adj, edge_attr, mask, W1, b1, W2, b2):
    x = np.ascontiguousarray(np.asarray(x, dtype=np.float32))
    adj = np.ascontiguousarray(np.asarray(adj, dtype=np.float32))
    edge_attr = np.ascontiguousarray(np.asarray(edge_attr, dtype=np.float32))
    mask = np.asarray(mask)
    W1 = np.ascontiguousarray(np.asarray(W1, dtype=np.float32))
    b1 = np.ascontiguousarray(np.asarray(b1, dtype=np.float32))
    W2 = np.ascontiguousarray(np.asarray(W2, dtype=np.float32))
    b2 = np.ascontiguousarray(np.asarray(b2, dtype=np.float32))

    # core c = 2*b + h: batch b, interleaved half h of b's kept nodes
    core_jj = []
    for b in range(B):
        jj = np.flatnonzero(mask[b])
        core_jj.append(jj[0::2])
        core_jj.append(jj[1::2])
    maxJ = max((len(jj) for jj in core_jj), default=1)
    Jp = max(JG, ((maxJ + JG - 1) // JG) * JG)

    nc = _get_prog(Jp)

    in_maps = []
    for c, jj in enumerate(core_jj):
        b = c // 2
        J = len(jj)
        edge_c = np.zeros((N, Jp, F), np.float32)
        if J:
            edge_c[:, :J] = edge_attr[b][:, jj, :]
        adj_c = np.zeros((N, Jp), np.float32)
        if J:
            adj_c[:, :J] = adj[b][:, jj]
        xk = np.zeros((Jp, F), np.float32)
        if J:
            xk[:J] = x[b][jj]
        adj_r = adj_c.reshape(NI, P, Jp).transpose(1, 0, 2).reshape(P, NI * Jp)
        xtr = np.concatenate([np.ones(JG, np.float32), xk.reshape(-1)])[None, :]
        cst = _pack_consts(Jp, W1, W2, b1, b2)
        x_r = x[b].reshape(NI, P, F).transpose(1, 0, 2).reshape(P, NI * F)
        in_maps.append({
            "edge": edge_c, "adj": np.ascontiguousarray(adj_r),
            "xtr": np.ascontiguousarray(xtr), "cst": cst,
            "x": np.ascontiguousarray(x_r),
        })

    res = run_bass_kernel_spmd(nc, in_maps, list(range(N_CORES)))

    out = np.zeros((B, N, F), np.float32)
    for c, jj in enumerate(core_jj):
        b = c // 2
        if len(jj):
            out[b][jj] = res.results[c]["out"][:len(jj)]
    return out
